# revision 12
# baseline (speedup 1.0000x reference)
"""Fused graph Fokker-Planck ODE function kernel for Trainium2 (8 NeuronCores).

Sharding: data-parallel over batch B=4 x row-halves (i in [0,256) / [256,512))
-> 8 shards.  Each core computes dh_dt for one (batch, i-half) pair.

Math (per batch; [i,j] matrices kept transposed as [j,i] on chip):
    S      = A * (K @ Q^T) / sqrt(D)       (elementwise mask, no -inf)
    X      = exp(S)                         (unnormalized softmax)
    sg     = sigmoid(10(E_j - E_i));  rd = 1 - sg;  M4 = X * rd
    Separable-sigmoid trick: sg = ez * rd with ez = a_j * b_i,
    a_j = e^{10 E_j}, b_i = e^{-10 E_i}.  Hence M3 = X*sg = diag(a) M4 diag(b)
    and   G3 = M3^T @ V = diag(b) * (M4^T @ (a x V))
    so ONE set of weights (M4) serves both accumulations:
      ppA[i, :] = M4^T @ [a*h | a*E*h | a*L*h | a | E | L | 1]   (131 cols)
    G3h = b*ppA[:,0:32], ..., r3 = b*ppA[:,96], G4E/G4L/r4 = ppA[:,97:131]
    s_i = r3 + r4;  dh as in the reference, assembled from these pieces.
"""

import math
import sys

import numpy as np

for _p in ("/opt/trn_rl_repo",):
    if _p not in sys.path:
        sys.path.insert(0, _p)

B, N, D, PED = 4, 512, 32, 16
NCORES = 8
RPC = N // 2            # i-rows per core
NJT = N // 128          # j tiles of 128
NIT = RPC // 128        # i tiles of 128
GW = 131                # columns per accumulation block
KSH = 10.0
ISD = 1.0 / math.sqrt(D)

_CACHE = {}


def _patch_act_tables():
    """Make natural_log_exp_and_others the only ACT table set containing our
    functions (exp/ln/identity/copy) so bacc emits exactly one
    ACT_TABLE_LOAD.  Dict length/order is preserved — the set INDEX is the
    runtime act_func_set_id, so entries must not be removed."""
    import concourse.bacc as bacc_mod
    if getattr(bacc_mod, "_act_tables_patched", False):
        return
    orig = bacc_mod.get_activation_tables

    def filtered(arch):
        t = orig(arch)
        target = t.get("natural_log_exp_and_others")
        if not target:
            return t
        return {k: (v if k == "natural_log_exp_and_others" else (v - target))
                for k, v in t.items()}

    bacc_mod.get_activation_tables = filtered
    bacc_mod._act_tables_patched = True


def _build_program():
    import concourse.bacc as bacc
    import concourse.tile as tile
    from concourse import mybir
    from contextlib import ExitStack

    _patch_act_tables()

    fp32 = mybir.dt.float32
    f32r = mybir.dt.float32r
    bf16 = mybir.dt.bfloat16
    AF = mybir.ActivationFunctionType
    ADD, MUL = mybir.AluOpType.add, mybir.AluOpType.mult
    SUB = mybir.AluOpType.subtract

    nc = bacc.Bacc("TRN2", target_bir_lowering=False, debug=False,
                   num_devices=NCORES)

    # ---------------- dram inputs ----------------
    peR = nc.dram_tensor("peR", [PED, 832], fp32, kind="ExternalInput").ap()
    smalls = nc.dram_tensor("smalls", [128, 20], fp32,
                            kind="ExternalInput").ap()
    at8 = nc.dram_tensor("at8", [128, NJT * RPC], fp32,
                         kind="ExternalInput").ap()
    big2 = nc.dram_tensor("big2", [128, 224], fp32,
                          kind="ExternalInput").ap()
    rdT = nc.dram_tensor("rdT", [128, NJT * RPC], bf16,
                         kind="ExternalInput").ap()
    out = nc.dram_tensor("out", [128, NIT * D], fp32,
                         kind="ExternalOutput").ap()

    with tile.TileContext(nc) as tc, ExitStack() as ctx:
        cst = ctx.enter_context(tc.tile_pool(name="cst", bufs=1))
        sb = ctx.enter_context(tc.tile_pool(name="sb", bufs=1))
        keep = ctx.enter_context(tc.tile_pool(name="keep", bufs=1))
        fin = ctx.enter_context(tc.tile_pool(name="fin", bufs=1))
        pq = ctx.enter_context(tc.tile_pool(name="pq", bufs=1, space="PSUM"))
        sps = ctx.enter_context(tc.tile_pool(name="sps", bufs=1, space="PSUM"))
        fps = ctx.enter_context(tc.tile_pool(name="fps", bufs=1, space="PSUM"))

        # ---------------- input DMAs (4 queues, issued first) ----------
        peR_sb = cst.tile([PED, 832], fp32, tag="peR_sb")
        nc.sync.dma_start(peR_sb[:, 320:832], peR[:, 320:832])
        nc.sync.dma_start(peR_sb[:, 0:320], peR[:, 0:320])
        smalls_sb = cst.tile([128, 20], fp32, tag="smalls_sb")
        nc.scalar.dma_start(smalls_sb[:], smalls[:])
        at_sb = cst.tile([128, NJT * RPC], fp32, tag="at_sb")
        nc.scalar.dma_start(at_sb[:], at8[:])
        big2_sb = cst.tile([128, 224], fp32, tag="big2_sb")
        nc.gpsimd.dma_start(big2_sb[:], big2[:])
        rd_sb = cst.tile([128, NJT * RPC], bf16, tag="rd_sb")
        nc.gpsimd.dma_start(rd_sb[:], rdT[:])

        # views
        ej_sb = smalls_sb[:, 0:NJT]            # E_j per (partition, jtile)
        ei_sb = smalls_sb[:, NJT:NJT + NIT]    # E_i per (partition, itile)
        bk_sb = smalls_sb[0:D, 6:7]
        bq_sb = smalls_sb[0:D, 7:8]
        hv = big2_sb[:, 0:128].rearrange("p (t d) -> p t d", d=D)
        hiv = big2_sb[:, 128:192].rearrange("p (t d) -> p t d", d=D)
        betab = big2_sb[:, 192:224]
        a_sb = smalls_sb[:, 8:12]              # e^{10 E_j}
        b_sb2 = smalls_sb[:, 12:14]            # e^{-10 E_i}
        aE_sb = smalls_sb[:, 14:18]            # a * E_j

        # ---------------- constants / ACT warm-up ----------------------
        zero1 = cst.tile([128, 1], fp32, tag="zero1")
        nc.vector.memset(zero1[:], 0.0)
        eps1 = cst.tile([128, 1], fp32, tag="eps1")
        nc.vector.memset(eps1[:], 1e-8)
        warm = cst.tile([128, 1], fp32, tag="warm")
        nc.scalar.activation(warm[:], zero1[:], AF.Exp, bias=zero1[:])

        # ---------------- casts (DVE) ----------------------------------
        pe2_r = cst.tile([PED, 512], f32r, tag="pe2_r")
        nc.vector.tensor_copy(pe2_r[:], peR_sb[:, 320:832])
        pe1_r = cst.tile([PED, 320], f32r, tag="pe1_r")
        nc.vector.tensor_copy(pe1_r[:], peR_sb[:, 0:320])

        peiT_r = pe1_r[:, 0:256]
        wk_r = pe1_r[:, 256:288]
        wq_r = pe1_r[:, 288:320]

        # ---------------- K / Q projections ----------------------------
        qps = pq.tile([D, N], fp32, tag="qps")
        nc.tensor.matmul(qps[:], wq_r, pe2_r[:], start=True, stop=True)
        qT = cst.tile([D, N], f32r, tag="qT")
        kps = pq.tile([D, RPC], fp32, tag="kps")
        nc.tensor.matmul(kps[:], wk_r, peiT_r, start=True, stop=True)
        kT = cst.tile([D, RPC], f32r, tag="kT")
        nc.vector.tensor_scalar(kT[:], kps[:], bk_sb, ISD, op0=ADD, op1=MUL)

        # ---------------- ACT early chain ------------------------------
        blk = keep.tile([128, NJT * GW], bf16, tag="blk")
        bv = blk.rearrange("p (t c) -> p t c", c=GW)
        nc.scalar.activation(bv[:, :, 98:130], hv[:], AF.Ln, bias=eps1[:])
        # qT = qps + bq (Identity), f32r out
        nc.scalar.activation(qT[:], qps[:], AF.Identity, bias=bq_sb, scale=1.0)

        # ---------------- blk prep (Pool) ------------------------------
        av = a_sb.rearrange("p (t o) -> p t o", o=1)
        ejv = ej_sb.rearrange("p (t o) -> p t o", o=1)
        aEv = aE_sb.rearrange("p (t o) -> p t o", o=1)
        one1 = cst.tile([128, 1], fp32, tag="one1")
        nc.vector.memset(one1[:], 1.0)
        nc.vector.tensor_tensor(bv[:, :, 0:32], hv[:],
                                av.to_broadcast((128, NJT, D)), op=MUL)
        nc.vector.tensor_tensor(bv[:, :, 32:64], hv[:],
                                aEv.to_broadcast((128, NJT, D)), op=MUL)
        nc.vector.tensor_tensor(bv[:, :, 64:96], bv[:, :, 98:130],
                                bv[:, :, 0:32], op=MUL)
        nc.vector.tensor_copy(bv[:, :, 96:97], av[:])
        nc.vector.tensor_copy(bv[:, :, 97:98], ejv[:])
        onev = one1.rearrange("p (t o) -> p t o", o=1)
        nc.vector.tensor_copy(bv[:, :, 130:131],
                              onev.to_broadcast((128, NJT, 1)))

        # ---------------- scores ---------------------------------------
        sall = sps.tile([128, NJT * RPC], fp32, tag="sall")
        for jt in range(NJT):
            nc.tensor.matmul(sall[:, jt * RPC:(jt + 1) * RPC],
                             qT[:, jt * 128:(jt + 1) * 128],
                             kT[:], start=True, stop=True)

        # ---------------- msk -> X -> M4 pipeline ----------------------
        msk = sb.tile([128, NJT * RPC], fp32, tag="msk")
        X = sb.tile([128, NJT * RPC], bf16, tag="X")
        M4 = keep.tile([128, NJT * RPC], bf16, tag="M4")

        def mskop(jt):
            sl = slice(jt * RPC, (jt + 1) * RPC)
            nc.vector.tensor_tensor(msk[:, sl], at_sb[:, sl], sall[:, sl],
                                    op=MUL)

        def xop(jt):
            sl = slice(jt * RPC, (jt + 1) * RPC)
            nc.scalar.activation(X[:, sl], msk[:, sl], AF.Exp, bias=zero1[:])

        def m4op(jt, eng):
            sl = slice(jt * RPC, (jt + 1) * RPC)
            eng.tensor_tensor(M4[:, sl], X[:, sl], rd_sb[:, sl], op=MUL)

        mskop(0)
        xop(0)
        mskop(1)
        m4op(0, nc.vector)
        xop(1)
        mskop(2)
        m4op(1, nc.vector)
        xop(2)
        mskop(3)
        m4op(2, nc.vector)
        xop(3)
        m4op(3, nc.vector)

        # ---------------- accumulation matmuls -------------------------
        # one accumulator per i-tile, each in its OWN 2KB PSUM bank —
        # interleaved open accumulation groups in one bank corrupt results
        BNK = 512
        ppA = fps.tile([128, NIT * BNK], fp32, tag="ppA")
        ppAv = ppA.rearrange("p (t c) -> p t c", c=BNK)
        for jt in range(NJT):
            st, sp = (jt == 0), (jt == NJT - 1)
            for it in range(NIT):
                nc.tensor.matmul(
                    ppA[:, it * BNK:it * BNK + GW],
                    M4[:, jt * RPC + it * 128:jt * RPC + (it + 1) * 128],
                    bv[:, jt, :], start=st, stop=sp)

        # ---------------- finals ---------------------------------------
        li = fin.tile([128, NIT, D], fp32, tag="li")
        nc.scalar.activation(li[:], hiv[:], AF.Ln, bias=eps1[:])

        # Pool cannot read PSUM: ACT copies the accumulators to SBUF (for
        # the Pool branch) while DVE starts immediately on PSUM-direct views.
        gA = fin.tile([128, NIT, GW], fp32, tag="gA")
        gAv2 = gA.rearrange("p t c -> p (t c)")
        nc.scalar.activation(gA[:], ppAv[:, :, 0:GW], AF.Identity,
                             bias=zero1[:], scale=1.0)
        # DVE branch reads PSUM directly; Pool branch reads the gA copy
        G3hr = ppAv[:, :, 0:D]
        G3Ehr = ppAv[:, :, D:2 * D]
        G3Lhr = gA[:, :, 0:D]          # Pool-side view of G3hr
        G3Lhr_p = gA[:, :, 2 * D:3 * D]
        r3r = ppAv[:, :, 96:97]
        G4E = ppAv[:, :, 97:98]
        G4L = gA[:, :, 98:130]
        r4 = ppAv[:, :, 130:131]
        r4_p = gA[:, :, 130:131]

        bvw = b_sb2.rearrange("p (t o) -> p t o", o=1)
        eivw = ei_sb.rearrange("p (t o) -> p t o", o=1)

        def bc(ap):
            return ap.to_broadcast((128, NIT, D))

        # row scalars
        r3b = fin.tile([128, NIT], fp32, tag="r3b")
        r3bv = r3b.rearrange("p (t o) -> p t o", o=1)
        nc.vector.tensor_tensor(r3bv[:], r3r, bvw[:], op=MUL)
        s_all = fin.tile([128, NIT], fp32, tag="s_all")
        svw = s_all.rearrange("p (t o) -> p t o", o=1)
        nc.vector.tensor_tensor(svw[:], r3bv[:], r4, op=ADD)
        invs = fin.tile([128, NIT], fp32, tag="invs")
        nc.vector.reciprocal(invs[:], s_all[:])
        m1 = fin.tile([128, NIT], fp32, tag="m1")
        m1v = m1.rearrange("p (t o) -> p t o", o=1)
        nc.vector.tensor_tensor(m1v[:], eivw[:], r4, op=MUL)
        u_all = fin.tile([128, NIT], fp32, tag="u_all")
        uv = u_all.rearrange("p (t o) -> p t o", o=1)
        nc.vector.tensor_sub(uv[:], G4E, m1v[:])

        # branch A (DVE): t12 = b*(G3Ehr - Ei*G3hr) + hi*u
        v1 = fin.tile([128, NIT, D], fp32, tag="v1")
        nc.vector.tensor_mul(v1[:], bc(eivw), G3hr)
        t1r = fin.tile([128, NIT, D], fp32, tag="t1r")
        nc.vector.tensor_sub(t1r[:], G3Ehr, v1[:])
        t1 = fin.tile([128, NIT, D], fp32, tag="t1")
        nc.vector.tensor_mul(t1[:], t1r[:], bc(bvw))
        v2 = fin.tile([128, NIT, D], fp32, tag="v2")
        nc.vector.tensor_mul(v2[:], hiv[:], bc(uv))
        t12 = fin.tile([128, NIT, D], fp32, tag="t12")
        nc.vector.tensor_add(t12[:], t1[:], v2[:])

        # branch B (Pool): e1 = b*(G3Lhr - Li*G3hr); q = hi*G4L; p2 = Li*r4*hi
        z1 = fin.tile([128, NIT, D], fp32, tag="z1")
        nc.gpsimd.tensor_tensor(z1[:], li[:], G3Lhr, op=MUL)
        e1r = fin.tile([128, NIT, D], fp32, tag="e1r")
        nc.gpsimd.tensor_tensor(e1r[:], G3Lhr_p, z1[:], op=SUB)
        e1 = fin.tile([128, NIT, D], fp32, tag="e1")
        nc.gpsimd.tensor_tensor(e1[:], e1r[:], bc(bvw), op=MUL)
        q = fin.tile([128, NIT, D], fp32, tag="q")
        nc.gpsimd.tensor_tensor(q[:], hiv[:], G4L, op=MUL)
        v3 = fin.tile([128, NIT, D], fp32, tag="v3")
        nc.gpsimd.tensor_tensor(v3[:], hiv[:], bc(r4_p), op=MUL)
        p2 = fin.tile([128, NIT, D], fp32, tag="p2")
        nc.gpsimd.tensor_tensor(p2[:], li[:], v3[:], op=MUL)

        # join (DVE)
        e2a = fin.tile([128, NIT, D], fp32, tag="e2a")
        nc.vector.tensor_add(e2a[:], e1[:], q[:])
        e2 = fin.tile([128, NIT, D], fp32, tag="e2")
        nc.vector.tensor_sub(e2[:], e2a[:], p2[:])
        bt = fin.tile([128, NIT, D], fp32, tag="bt")
        bbv = betab.rearrange("p (t d) -> p t d", t=1).to_broadcast(
            (128, NIT, D))
        nc.vector.tensor_mul(bt[:], e2[:], bbv)
        pre = fin.tile([128, NIT, D], fp32, tag="pre")
        nc.vector.tensor_add(pre[:], t12[:], bt[:])
        res = fin.tile([128, NIT, D], fp32, tag="res")
        iv = invs.rearrange("p (t o) -> p t o", o=1)
        nc.vector.tensor_mul(res[:], pre[:], bc(iv))
        nc.sync.dma_start(out[:], res.rearrange("p t d -> p (t d)"))

    nc.compile()
    return nc


def _get_program():
    if "nc" not in _CACHE:
        _CACHE["nc"] = _build_program()
    return _CACHE["nc"]


def make_in_maps(h, pe, E, A, Wk, bk, Wq, bq, beta):
    import ml_dtypes
    f = lambda x: np.ascontiguousarray(np.asarray(x, dtype=np.float32))
    h, pe, E, A = f(h), f(pe), f(E), f(A)
    Wk, bk, Wq, bq, beta = f(Wk), f(bk), f(Wq), f(bq), f(beta)
    in_maps = []
    for c in range(NCORES):
        b, r = c // 2, c % 2
        isl = slice(r * RPC, (r + 1) * RPC)
        smalls = np.zeros((128, 20), np.float32)
        smalls[:, 0:NJT] = E.reshape(NJT, 128).T
        smalls[:, NJT:NJT + NIT] = E[isl].reshape(NIT, 128).T
        smalls[0:D, 6] = bk
        smalls[0:D, 7] = bq
        smalls[:, 8:12] = np.exp(KSH * E).reshape(NJT, 128).T
        smalls[:, 12:14] = np.exp(-KSH * E[isl]).reshape(NIT, 128).T
        smalls[:, 14:18] = (np.exp(KSH * E) * E).reshape(NJT, 128).T
        peR = np.zeros((PED, 832), np.float32)
        peR[:, 0:256] = pe[b, isl].T
        peR[:, 256:288] = Wk
        peR[:, 288:320] = Wq
        peR[:, 320:832] = pe[b].T
        atp = A[isl].T.reshape(NJT, 128, RPC).transpose(1, 0, 2)
        at8 = f(atp.reshape(128, NJT * RPC))
        hjp = h[b].reshape(NJT, 128, D).transpose(1, 0, 2)
        hip = h[b, isl].reshape(NIT, 128, D).transpose(1, 0, 2)
        big2 = np.zeros((128, 224), np.float32)
        big2[:, 0:128] = hjp.reshape(128, NJT * D)
        big2[:, 128:192] = hip.reshape(128, NIT * D)
        big2[:, 192:224] = np.broadcast_to(beta, (128, D))
        # rd[j, i] = 1 - sigmoid(10 (E_j - E_i)), layout [p, (t i)] like AT
        ezt = np.exp(KSH * (E[:, None] - E[None, isl]))      # [j, i]
        rdf = (1.0 / (1.0 + ezt)).astype(np.float32)
        rdp = rdf.reshape(NJT, 128, RPC).transpose(1, 0, 2)
        rdT = np.ascontiguousarray(
            rdp.reshape(128, NJT * RPC)).astype(ml_dtypes.bfloat16)
        in_maps.append({
            "peR": peR,
            "smalls": smalls,
            "at8": at8,
            "big2": big2,
            "rdT": rdT,
        })
    return in_maps


def gather(results):
    out = np.empty((B, N, D), np.float32)
    for c in range(NCORES):
        b, r = c // 2, c % 2
        o = results[c]["out"].reshape(128, NIT, D).transpose(1, 0, 2)
        out[b, r * RPC:(r + 1) * RPC] = o.reshape(RPC, D)
    return out


def _axon_reset():
    try:
        import ctypes
        import jax
        lib = ctypes.CDLL("/opt/axon/libaxon_pjrt.so")
        lib.axon_reset.restype = ctypes.c_int64
        jax.devices()
        lib.axon_reset()
    except Exception:
        pass


def kernel(t=None, h=None, pe=None, E=None, A=None, Wk=None, bk=None,
           Wq=None, bq=None, beta=None, **_unused):
    from concourse.bass_utils import run_bass_kernel_spmd
    nc = _get_program()
    in_maps = make_in_maps(h, pe, E, A, Wk, bk, Wq, bq, beta)
    try:
        res = run_bass_kernel_spmd(nc, in_maps, list(range(NCORES)))
    except Exception:
        # a previously wedged NeuronCore shows up as an opaque runtime
        # error on the first execute — reset the device once and retry
        _axon_reset()
        import time as _time
        _time.sleep(2)
        res = run_bass_kernel_spmd(nc, in_maps, list(range(NCORES)))
    return gather(res.results)


# revision 13
# speedup vs baseline: 1.0146x; 1.0146x over previous
"""Fused graph Fokker-Planck ODE function kernel for Trainium2 (8 NeuronCores).

Sharding: data-parallel over batch B=4 x row-halves (i in [0,256) / [256,512))
-> 8 shards.  Each core computes dh_dt for one (batch, i-half) pair.

Math (per batch; [i,j] matrices kept transposed as [j,i] on chip):
    S      = A * (K @ Q^T) / sqrt(D)       (elementwise mask, no -inf)
    X      = exp(S)                         (unnormalized softmax)
    sg     = sigmoid(10(E_j - E_i));  rd = 1 - sg;  M4 = X * rd
    Separable-sigmoid trick: sg = ez * rd with ez = a_j * b_i,
    a_j = e^{10 E_j}, b_i = e^{-10 E_i}.  Hence M3 = X*sg = diag(a) M4 diag(b)
    and   G3 = M3^T @ V = diag(b) * (M4^T @ (a x V))
    so ONE set of weights (M4) serves both accumulations:
      ppA[i, :] = M4^T @ [a*h | a*E*h | a*L*h | a | E | L | 1]   (131 cols)
    G3h = b*ppA[:,0:32], ..., r3 = b*ppA[:,96], G4E/G4L/r4 = ppA[:,97:131]
    s_i = r3 + r4;  dh as in the reference, assembled from these pieces.
"""

import math
import sys

import numpy as np

for _p in ("/opt/trn_rl_repo",):
    if _p not in sys.path:
        sys.path.insert(0, _p)

B, N, D, PED = 4, 512, 32, 16
NCORES = 8
RPC = N // 2            # i-rows per core
NJT = N // 128          # j tiles of 128
NIT = RPC // 128        # i tiles of 128
GW = 131                # columns per accumulation block
KSH = 10.0
ISD = 1.0 / math.sqrt(D)

_CACHE = {}


def _patch_act_tables():
    """Make natural_log_exp_and_others the only ACT table set containing our
    functions (exp/ln/identity/copy) so bacc emits exactly one
    ACT_TABLE_LOAD.  Dict length/order is preserved — the set INDEX is the
    runtime act_func_set_id, so entries must not be removed."""
    import concourse.bacc as bacc_mod
    if getattr(bacc_mod, "_act_tables_patched", False):
        return
    orig = bacc_mod.get_activation_tables

    def filtered(arch):
        t = orig(arch)
        target = t.get("natural_log_exp_and_others")
        if not target:
            return t
        return {k: (v if k == "natural_log_exp_and_others" else (v - target))
                for k, v in t.items()}

    bacc_mod.get_activation_tables = filtered
    bacc_mod._act_tables_patched = True


def _build_program():
    import concourse.bacc as bacc
    import concourse.tile as tile
    from concourse import mybir
    from contextlib import ExitStack

    _patch_act_tables()

    fp32 = mybir.dt.float32
    f32r = mybir.dt.float32r
    bf16 = mybir.dt.bfloat16
    AF = mybir.ActivationFunctionType
    ADD, MUL = mybir.AluOpType.add, mybir.AluOpType.mult
    SUB = mybir.AluOpType.subtract

    nc = bacc.Bacc("TRN2", target_bir_lowering=False, debug=False,
                   num_devices=NCORES)

    # ---------------- dram inputs ----------------
    peR = nc.dram_tensor("peR", [PED, 832], fp32, kind="ExternalInput").ap()
    smalls = nc.dram_tensor("smalls", [128, 20], fp32,
                            kind="ExternalInput").ap()
    at8 = nc.dram_tensor("at8", [128, NJT * RPC], fp32,
                         kind="ExternalInput").ap()
    big2 = nc.dram_tensor("big2", [128, 224], fp32,
                          kind="ExternalInput").ap()
    rdT = nc.dram_tensor("rdT", [128, NJT * RPC], bf16,
                         kind="ExternalInput").ap()
    out = nc.dram_tensor("out", [128, NIT * D], fp32,
                         kind="ExternalOutput").ap()

    with tile.TileContext(nc) as tc, ExitStack() as ctx:
        cst = ctx.enter_context(tc.tile_pool(name="cst", bufs=1))
        sb = ctx.enter_context(tc.tile_pool(name="sb", bufs=1))
        keep = ctx.enter_context(tc.tile_pool(name="keep", bufs=1))
        fin = ctx.enter_context(tc.tile_pool(name="fin", bufs=1))
        pq = ctx.enter_context(tc.tile_pool(name="pq", bufs=1, space="PSUM"))
        sps = ctx.enter_context(tc.tile_pool(name="sps", bufs=1, space="PSUM"))
        fps = ctx.enter_context(tc.tile_pool(name="fps", bufs=1, space="PSUM"))

        # ---------------- input DMAs (4 queues, issued first) ----------
        peR_sb = cst.tile([PED, 832], fp32, tag="peR_sb")
        nc.sync.dma_start(peR_sb[:, 0:576], peR[:, 0:576])
        nc.sync.dma_start(peR_sb[:, 576:832], peR[:, 576:832])
        smalls_sb = cst.tile([128, 20], fp32, tag="smalls_sb")
        nc.scalar.dma_start(smalls_sb[:], smalls[:])
        at_sb = cst.tile([128, NJT * RPC], fp32, tag="at_sb")
        nc.scalar.dma_start(at_sb[:], at8[:])
        big2_sb = cst.tile([128, 224], fp32, tag="big2_sb")
        nc.gpsimd.dma_start(big2_sb[:], big2[:])
        rd_sb = cst.tile([128, NJT * RPC], bf16, tag="rd_sb")
        nc.gpsimd.dma_start(rd_sb[:], rdT[:])

        # views
        ej_sb = smalls_sb[:, 0:NJT]            # E_j per (partition, jtile)
        ei_sb = smalls_sb[:, NJT:NJT + NIT]    # E_i per (partition, itile)
        bk_sb = smalls_sb[0:D, 6:7]
        bq_sb = smalls_sb[0:D, 7:8]
        hv = big2_sb[:, 0:128].rearrange("p (t d) -> p t d", d=D)
        hiv = big2_sb[:, 128:192].rearrange("p (t d) -> p t d", d=D)
        betab = big2_sb[:, 192:224]
        a_sb = smalls_sb[:, 8:12]              # e^{10 E_j}
        b_sb2 = smalls_sb[:, 12:14]            # e^{-10 E_i}
        aE_sb = smalls_sb[:, 14:18]            # a * E_j

        # ---------------- constants / ACT warm-up ----------------------
        zero1 = cst.tile([128, 1], fp32, tag="zero1")
        nc.vector.memset(zero1[:], 0.0)
        eps1 = cst.tile([128, 1], fp32, tag="eps1")
        nc.vector.memset(eps1[:], 1e-8)
        warm = cst.tile([128, 1], fp32, tag="warm")
        nc.scalar.activation(warm[:], zero1[:], AF.Exp, bias=zero1[:])

        # ---------------- casts (DVE) ----------------------------------
        pe1_r = cst.tile([PED, 576], f32r, tag="pe1_r")
        nc.vector.tensor_copy(pe1_r[:], peR_sb[:, 0:576])
        pe2_r = cst.tile([PED, 256], f32r, tag="pe2_r")
        nc.vector.tensor_copy(pe2_r[:], peR_sb[:, 576:832])

        peT_r = pe1_r[:, 0:512]
        wk_r = pe1_r[:, 512:544]
        wq_r = pe1_r[:, 544:576]
        peiT_r = pe2_r[:]

        # ---------------- K / Q projections ----------------------------
        qps = pq.tile([D, N], fp32, tag="qps")
        nc.tensor.matmul(qps[:], wq_r, peT_r, start=True, stop=True)
        qT = cst.tile([D, N], f32r, tag="qT")
        kps = pq.tile([D, RPC], fp32, tag="kps")
        nc.tensor.matmul(kps[:], wk_r, peiT_r, start=True, stop=True)
        kT = cst.tile([D, RPC], f32r, tag="kT")
        nc.vector.tensor_scalar(kT[:], kps[:], bk_sb, ISD, op0=ADD, op1=MUL)

        # ---------------- ACT early chain ------------------------------
        blk = keep.tile([128, NJT * GW], bf16, tag="blk")
        bv = blk.rearrange("p (t c) -> p t c", c=GW)
        nc.scalar.activation(bv[:, :, 98:130], hv[:], AF.Ln, bias=eps1[:])
        # qT = qps + bq (Identity), f32r out
        nc.scalar.activation(qT[:], qps[:], AF.Identity, bias=bq_sb, scale=1.0)

        # ---------------- blk prep (Pool) ------------------------------
        av = a_sb.rearrange("p (t o) -> p t o", o=1)
        ejv = ej_sb.rearrange("p (t o) -> p t o", o=1)
        aEv = aE_sb.rearrange("p (t o) -> p t o", o=1)
        one1 = cst.tile([128, 1], fp32, tag="one1")
        nc.vector.memset(one1[:], 1.0)
        nc.vector.tensor_tensor(bv[:, :, 0:32], hv[:],
                                av.to_broadcast((128, NJT, D)), op=MUL)
        nc.vector.tensor_tensor(bv[:, :, 32:64], hv[:],
                                aEv.to_broadcast((128, NJT, D)), op=MUL)
        nc.vector.tensor_tensor(bv[:, :, 64:96], bv[:, :, 98:130],
                                bv[:, :, 0:32], op=MUL)
        nc.vector.tensor_copy(bv[:, :, 96:97], av[:])
        nc.vector.tensor_copy(bv[:, :, 97:98], ejv[:])
        onev = one1.rearrange("p (t o) -> p t o", o=1)
        nc.vector.tensor_copy(bv[:, :, 130:131],
                              onev.to_broadcast((128, NJT, 1)))

        # ---------------- scores ---------------------------------------
        sall = sps.tile([128, NJT * RPC], fp32, tag="sall")
        for jt in range(NJT):
            nc.tensor.matmul(sall[:, jt * RPC:(jt + 1) * RPC],
                             qT[:, jt * 128:(jt + 1) * 128],
                             kT[:], start=True, stop=True)

        # ---------------- msk -> X -> M4 pipeline ----------------------
        msk = sb.tile([128, NJT * RPC], fp32, tag="msk")
        X = sb.tile([128, NJT * RPC], bf16, tag="X")
        M4 = keep.tile([128, NJT * RPC], bf16, tag="M4")

        def mskop(jt):
            sl = slice(jt * RPC, (jt + 1) * RPC)
            nc.vector.tensor_tensor(msk[:, sl], at_sb[:, sl], sall[:, sl],
                                    op=MUL)

        def xop(jt):
            sl = slice(jt * RPC, (jt + 1) * RPC)
            nc.scalar.activation(X[:, sl], msk[:, sl], AF.Exp, bias=zero1[:])

        def m4op(jt, eng):
            sl = slice(jt * RPC, (jt + 1) * RPC)
            eng.tensor_tensor(M4[:, sl], X[:, sl], rd_sb[:, sl], op=MUL)

        mskop(0)
        xop(0)
        mskop(1)
        m4op(0, nc.vector)
        xop(1)
        mskop(2)
        m4op(1, nc.vector)
        xop(2)
        mskop(3)
        m4op(2, nc.vector)
        xop(3)
        m4op(3, nc.vector)

        # ---------------- accumulation matmuls -------------------------
        # one accumulator per i-tile, each in its OWN 2KB PSUM bank —
        # interleaved open accumulation groups in one bank corrupt results
        BNK = 512
        ppA = fps.tile([128, NIT * BNK], fp32, tag="ppA")
        ppAv = ppA.rearrange("p (t c) -> p t c", c=BNK)
        for jt in range(NJT):
            st, sp = (jt == 0), (jt == NJT - 1)
            for it in range(NIT):
                nc.tensor.matmul(
                    ppA[:, it * BNK:it * BNK + GW],
                    M4[:, jt * RPC + it * 128:jt * RPC + (it + 1) * 128],
                    bv[:, jt, :], start=st, stop=sp)

        # ---------------- finals ---------------------------------------
        li = fin.tile([128, NIT, D], fp32, tag="li")
        nc.scalar.activation(li[:], hiv[:], AF.Ln, bias=eps1[:])

        # Pool cannot read PSUM: ACT copies the accumulators to SBUF (for
        # the Pool branch) while DVE starts immediately on PSUM-direct views.
        gA = fin.tile([128, NIT, GW], fp32, tag="gA")
        gAv2 = gA.rearrange("p t c -> p (t c)")
        nc.scalar.activation(gA[:], ppAv[:, :, 0:GW], AF.Identity,
                             bias=zero1[:], scale=1.0)
        # DVE branch reads PSUM directly; Pool branch reads the gA copy
        G3hr = ppAv[:, :, 0:D]
        G3Ehr = ppAv[:, :, D:2 * D]
        G3Lhr = gA[:, :, 0:D]          # Pool-side view of G3hr
        G3Lhr_p = gA[:, :, 2 * D:3 * D]
        r3r = ppAv[:, :, 96:97]
        G4E = ppAv[:, :, 97:98]
        G4L = gA[:, :, 98:130]
        r4 = ppAv[:, :, 130:131]
        r4_p = gA[:, :, 130:131]

        bvw = b_sb2.rearrange("p (t o) -> p t o", o=1)
        eivw = ei_sb.rearrange("p (t o) -> p t o", o=1)

        def bc(ap):
            return ap.to_broadcast((128, NIT, D))

        # row scalars
        r3b = fin.tile([128, NIT], fp32, tag="r3b")
        r3bv = r3b.rearrange("p (t o) -> p t o", o=1)
        nc.vector.tensor_tensor(r3bv[:], r3r, bvw[:], op=MUL)
        s_all = fin.tile([128, NIT], fp32, tag="s_all")
        svw = s_all.rearrange("p (t o) -> p t o", o=1)
        nc.vector.tensor_tensor(svw[:], r3bv[:], r4, op=ADD)
        invs = fin.tile([128, NIT], fp32, tag="invs")
        nc.vector.reciprocal(invs[:], s_all[:])
        m1 = fin.tile([128, NIT], fp32, tag="m1")
        m1v = m1.rearrange("p (t o) -> p t o", o=1)
        nc.vector.tensor_tensor(m1v[:], eivw[:], r4, op=MUL)
        u_all = fin.tile([128, NIT], fp32, tag="u_all")
        uv = u_all.rearrange("p (t o) -> p t o", o=1)
        nc.vector.tensor_sub(uv[:], G4E, m1v[:])

        # branch A (DVE): t12 = b*(G3Ehr - Ei*G3hr) + hi*u
        v1 = fin.tile([128, NIT, D], fp32, tag="v1")
        nc.vector.tensor_mul(v1[:], bc(eivw), G3hr)
        t1r = fin.tile([128, NIT, D], fp32, tag="t1r")
        nc.vector.tensor_sub(t1r[:], G3Ehr, v1[:])
        t1 = fin.tile([128, NIT, D], fp32, tag="t1")
        nc.vector.tensor_mul(t1[:], t1r[:], bc(bvw))
        v2 = fin.tile([128, NIT, D], fp32, tag="v2")
        nc.vector.tensor_mul(v2[:], hiv[:], bc(uv))
        t12 = fin.tile([128, NIT, D], fp32, tag="t12")
        nc.vector.tensor_add(t12[:], t1[:], v2[:])

        # branch B (Pool): e1 = b*(G3Lhr - Li*G3hr); q = hi*G4L; p2 = Li*r4*hi
        z1 = fin.tile([128, NIT, D], fp32, tag="z1")
        nc.gpsimd.tensor_tensor(z1[:], li[:], G3Lhr, op=MUL)
        e1r = fin.tile([128, NIT, D], fp32, tag="e1r")
        nc.gpsimd.tensor_tensor(e1r[:], G3Lhr_p, z1[:], op=SUB)
        e1 = fin.tile([128, NIT, D], fp32, tag="e1")
        nc.gpsimd.tensor_tensor(e1[:], e1r[:], bc(bvw), op=MUL)
        q = fin.tile([128, NIT, D], fp32, tag="q")
        nc.gpsimd.tensor_tensor(q[:], hiv[:], G4L, op=MUL)
        v3 = fin.tile([128, NIT, D], fp32, tag="v3")
        nc.gpsimd.tensor_tensor(v3[:], hiv[:], bc(r4_p), op=MUL)
        p2 = fin.tile([128, NIT, D], fp32, tag="p2")
        nc.gpsimd.tensor_tensor(p2[:], li[:], v3[:], op=MUL)

        # join (DVE)
        e2a = fin.tile([128, NIT, D], fp32, tag="e2a")
        nc.vector.tensor_add(e2a[:], e1[:], q[:])
        e2 = fin.tile([128, NIT, D], fp32, tag="e2")
        nc.vector.tensor_sub(e2[:], e2a[:], p2[:])
        bt = fin.tile([128, NIT, D], fp32, tag="bt")
        bbv = betab.rearrange("p (t d) -> p t d", t=1).to_broadcast(
            (128, NIT, D))
        nc.vector.tensor_mul(bt[:], e2[:], bbv)
        pre = fin.tile([128, NIT, D], fp32, tag="pre")
        nc.vector.tensor_add(pre[:], t12[:], bt[:])
        res = fin.tile([128, NIT, D], fp32, tag="res")
        iv = invs.rearrange("p (t o) -> p t o", o=1)
        nc.vector.tensor_mul(res[:], pre[:], bc(iv))
        nc.sync.dma_start(out[:], res.rearrange("p t d -> p (t d)"))

    nc.compile()
    return nc


def _get_program():
    if "nc" not in _CACHE:
        _CACHE["nc"] = _build_program()
    return _CACHE["nc"]


def make_in_maps(h, pe, E, A, Wk, bk, Wq, bq, beta):
    import ml_dtypes
    f = lambda x: np.ascontiguousarray(np.asarray(x, dtype=np.float32))
    h, pe, E, A = f(h), f(pe), f(E), f(A)
    Wk, bk, Wq, bq, beta = f(Wk), f(bk), f(Wq), f(bq), f(beta)
    in_maps = []
    for c in range(NCORES):
        b, r = c // 2, c % 2
        isl = slice(r * RPC, (r + 1) * RPC)
        smalls = np.zeros((128, 20), np.float32)
        smalls[:, 0:NJT] = E.reshape(NJT, 128).T
        smalls[:, NJT:NJT + NIT] = E[isl].reshape(NIT, 128).T
        smalls[0:D, 6] = bk
        smalls[0:D, 7] = bq
        smalls[:, 8:12] = np.exp(KSH * E).reshape(NJT, 128).T
        smalls[:, 12:14] = np.exp(-KSH * E[isl]).reshape(NIT, 128).T
        smalls[:, 14:18] = (np.exp(KSH * E) * E).reshape(NJT, 128).T
        peR = np.zeros((PED, 832), np.float32)
        peR[:, 0:512] = pe[b].T
        peR[:, 512:544] = Wk
        peR[:, 544:576] = Wq
        peR[:, 576:832] = pe[b, isl].T
        atp = A[isl].T.reshape(NJT, 128, RPC).transpose(1, 0, 2)
        at8 = f(atp.reshape(128, NJT * RPC))
        hjp = h[b].reshape(NJT, 128, D).transpose(1, 0, 2)
        hip = h[b, isl].reshape(NIT, 128, D).transpose(1, 0, 2)
        big2 = np.zeros((128, 224), np.float32)
        big2[:, 0:128] = hjp.reshape(128, NJT * D)
        big2[:, 128:192] = hip.reshape(128, NIT * D)
        big2[:, 192:224] = np.broadcast_to(beta, (128, D))
        # rd[j, i] = 1 - sigmoid(10 (E_j - E_i)), layout [p, (t i)] like AT
        ezt = np.exp(KSH * (E[:, None] - E[None, isl]))      # [j, i]
        rdf = (1.0 / (1.0 + ezt)).astype(np.float32)
        rdp = rdf.reshape(NJT, 128, RPC).transpose(1, 0, 2)
        rdT = np.ascontiguousarray(
            rdp.reshape(128, NJT * RPC)).astype(ml_dtypes.bfloat16)
        in_maps.append({
            "peR": peR,
            "smalls": smalls,
            "at8": at8,
            "big2": big2,
            "rdT": rdT,
        })
    return in_maps


def gather(results):
    out = np.empty((B, N, D), np.float32)
    for c in range(NCORES):
        b, r = c // 2, c % 2
        o = results[c]["out"].reshape(128, NIT, D).transpose(1, 0, 2)
        out[b, r * RPC:(r + 1) * RPC] = o.reshape(RPC, D)
    return out


def _axon_reset():
    try:
        import ctypes
        import jax
        lib = ctypes.CDLL("/opt/axon/libaxon_pjrt.so")
        lib.axon_reset.restype = ctypes.c_int64
        jax.devices()
        lib.axon_reset()
    except Exception:
        pass


def kernel(t=None, h=None, pe=None, E=None, A=None, Wk=None, bk=None,
           Wq=None, bq=None, beta=None, **_unused):
    from concourse.bass_utils import run_bass_kernel_spmd
    nc = _get_program()
    in_maps = make_in_maps(h, pe, E, A, Wk, bk, Wq, bq, beta)
    try:
        res = run_bass_kernel_spmd(nc, in_maps, list(range(NCORES)))
    except Exception:
        # a previously wedged NeuronCore shows up as an opaque runtime
        # error on the first execute — reset the device once and retry
        _axon_reset()
        import time as _time
        _time.sleep(2)
        res = run_bass_kernel_spmd(nc, in_maps, list(range(NCORES)))
    return gather(res.results)


# revision 15
# speedup vs baseline: 1.0517x; 1.0366x over previous
"""Fused graph Fokker-Planck ODE function kernel for Trainium2 (8 NeuronCores).

Sharding: data-parallel over batch B=4 x row-halves (i in [0,256) / [256,512))
-> 8 shards.  Each core computes dh_dt for one (batch, i-half) pair.

Math (per batch; [i,j] matrices kept transposed as [j,i] on chip):
    S      = A * (K @ Q^T) / sqrt(D)       (elementwise mask, no -inf)
    X      = exp(S)                         (unnormalized softmax)
    sg     = sigmoid(10(E_j - E_i));  rd = 1 - sg;  M4 = X * rd
    Separable-sigmoid trick: sg = ez * rd with ez = a_j * b_i,
    a_j = e^{10 E_j}, b_i = e^{-10 E_i}.  Hence M3 = X*sg = diag(a) M4 diag(b)
    and   G3 = M3^T @ V = diag(b) * (M4^T @ (a x V))
    so ONE set of weights (M4) serves both accumulations:
      ppA[i, :] = M4^T @ [a*h | a*E*h | a*L*h | a | E | L | 1]   (131 cols)
    G3h = b*ppA[:,0:32], ..., r3 = b*ppA[:,96], G4E/G4L/r4 = ppA[:,97:131]
    s_i = r3 + r4;  dh as in the reference, assembled from these pieces.
"""

import math
import sys

import numpy as np

for _p in ("/opt/trn_rl_repo",):
    if _p not in sys.path:
        sys.path.insert(0, _p)

B, N, D, PED = 4, 512, 32, 16
NCORES = 8
RPC = N // 2            # i-rows per core
NJT = N // 128          # j tiles of 128
NIT = RPC // 128        # i tiles of 128
GW = 131                # columns per accumulation block
KSH = 10.0
ISD = 1.0 / math.sqrt(D)

_CACHE = {}


def _patch_act_tables():
    """Make natural_log_exp_and_others the only ACT table set containing our
    functions (exp/ln/identity/copy) so bacc emits exactly one
    ACT_TABLE_LOAD.  Dict length/order is preserved — the set INDEX is the
    runtime act_func_set_id, so entries must not be removed."""
    import concourse.bacc as bacc_mod
    if getattr(bacc_mod, "_act_tables_patched", False):
        return
    orig = bacc_mod.get_activation_tables

    def filtered(arch):
        t = orig(arch)
        target = t.get("natural_log_exp_and_others")
        if not target:
            return t
        return {k: (v if k == "natural_log_exp_and_others" else (v - target))
                for k, v in t.items()}

    bacc_mod.get_activation_tables = filtered
    bacc_mod._act_tables_patched = True


def _build_program():
    import concourse.bacc as bacc
    import concourse.tile as tile
    from concourse import mybir
    from contextlib import ExitStack

    _patch_act_tables()

    fp32 = mybir.dt.float32
    f32r = mybir.dt.float32r
    bf16 = mybir.dt.bfloat16
    AF = mybir.ActivationFunctionType
    ADD, MUL = mybir.AluOpType.add, mybir.AluOpType.mult
    SUB = mybir.AluOpType.subtract

    nc = bacc.Bacc("TRN2", target_bir_lowering=False, debug=False,
                   num_devices=NCORES)

    # ---------------- dram inputs ----------------
    peR = nc.dram_tensor("peR", [PED, 832], fp32, kind="ExternalInput").ap()
    smalls = nc.dram_tensor("smalls", [128, 20], fp32,
                            kind="ExternalInput").ap()
    at8 = nc.dram_tensor("at8", [128, NJT * RPC], fp32,
                         kind="ExternalInput").ap()
    big2 = nc.dram_tensor("big2", [128, 224], fp32,
                          kind="ExternalInput").ap()
    rdT = nc.dram_tensor("rdT", [128, NJT * RPC], bf16,
                         kind="ExternalInput").ap()
    out = nc.dram_tensor("out", [128, NIT * D], fp32,
                         kind="ExternalOutput").ap()

    with tile.TileContext(nc) as tc, ExitStack() as ctx:
        cst = ctx.enter_context(tc.tile_pool(name="cst", bufs=1))
        sb = ctx.enter_context(tc.tile_pool(name="sb", bufs=1))
        keep = ctx.enter_context(tc.tile_pool(name="keep", bufs=1))
        fin = ctx.enter_context(tc.tile_pool(name="fin", bufs=1))
        pq = ctx.enter_context(tc.tile_pool(name="pq", bufs=1, space="PSUM"))
        sps = ctx.enter_context(tc.tile_pool(name="sps", bufs=1, space="PSUM"))
        fps = ctx.enter_context(tc.tile_pool(name="fps", bufs=1, space="PSUM"))

        # ---------------- input DMAs (4 queues, issued first) ----------
        peR_sb = cst.tile([PED, 832], fp32, tag="peR_sb")
        nc.sync.dma_start(peR_sb[:, 0:576], peR[:, 0:576])
        nc.sync.dma_start(peR_sb[:, 576:832], peR[:, 576:832])
        smalls_sb = cst.tile([128, 20], fp32, tag="smalls_sb")
        nc.scalar.dma_start(smalls_sb[:], smalls[:])
        at_sb = cst.tile([128, NJT * RPC], fp32, tag="at_sb")
        nc.scalar.dma_start(at_sb[:], at8[:])
        big2_sb = cst.tile([128, 224], fp32, tag="big2_sb")
        nc.gpsimd.dma_start(big2_sb[:], big2[:])
        rd_sb = cst.tile([128, NJT * RPC], bf16, tag="rd_sb")
        nc.gpsimd.dma_start(rd_sb[:], rdT[:])

        # views
        ej_sb = smalls_sb[:, 0:NJT]            # E_j per (partition, jtile)
        ei_sb = smalls_sb[:, NJT:NJT + NIT]    # E_i per (partition, itile)
        bk_sb = smalls_sb[0:D, 6:7]
        bq_sb = smalls_sb[0:D, 7:8]
        hv = big2_sb[:, 0:128].rearrange("p (t d) -> p t d", d=D)
        hiv = big2_sb[:, 128:192].rearrange("p (t d) -> p t d", d=D)
        betab = big2_sb[:, 192:224]
        a_sb = smalls_sb[:, 8:12]              # e^{10 E_j}
        b_sb2 = smalls_sb[:, 12:14]            # e^{-10 E_i}
        aE_sb = smalls_sb[:, 14:18]            # a * E_j

        # ---------------- constants / ACT warm-up ----------------------
        zero1 = cst.tile([128, 1], fp32, tag="zero1")
        nc.vector.memset(zero1[:], 0.0)
        eps1 = cst.tile([128, 1], fp32, tag="eps1")
        nc.vector.memset(eps1[:], 1e-8)
        warm = cst.tile([128, 1], fp32, tag="warm")
        nc.scalar.activation(warm[:], zero1[:], AF.Exp, bias=zero1[:])

        # ---------------- casts (DVE) ----------------------------------
        pe1_r = cst.tile([PED, 576], f32r, tag="pe1_r")
        nc.vector.tensor_copy(pe1_r[:], peR_sb[:, 0:576])
        pe2_r = cst.tile([PED, 256], f32r, tag="pe2_r")
        nc.vector.tensor_copy(pe2_r[:], peR_sb[:, 576:832])

        peT_r = pe1_r[:, 0:512]
        wk_r = pe1_r[:, 512:544]
        wq_r = pe1_r[:, 544:576]
        peiT_r = pe2_r[:]

        # ---------------- K / Q projections ----------------------------
        qps = pq.tile([D, N], fp32, tag="qps")
        nc.tensor.matmul(qps[:], wq_r, peT_r, start=True, stop=True)
        qT = cst.tile([D, N], f32r, tag="qT")
        kps = pq.tile([D, RPC], fp32, tag="kps")
        nc.tensor.matmul(kps[:], wk_r, peiT_r, start=True, stop=True)
        kT = cst.tile([D, RPC], f32r, tag="kT")
        nc.vector.tensor_scalar(kT[:], kps[:], bk_sb, ISD, op0=ADD, op1=MUL)

        # ---------------- ACT early chain ------------------------------
        blk = keep.tile([128, NJT * GW], bf16, tag="blk")
        bv = blk.rearrange("p (t c) -> p t c", c=GW)
        nc.scalar.activation(bv[:, :, 98:130], hv[:], AF.Ln, bias=eps1[:])
        # qT = qps + bq (Identity), f32r out
        nc.scalar.activation(qT[:], qps[:], AF.Identity, bias=bq_sb, scale=1.0)

        # ---------------- blk prep (Pool) ------------------------------
        av = a_sb.rearrange("p (t o) -> p t o", o=1)
        ejv = ej_sb.rearrange("p (t o) -> p t o", o=1)
        aEv = aE_sb.rearrange("p (t o) -> p t o", o=1)
        one1 = cst.tile([128, 1], fp32, tag="one1")
        nc.vector.memset(one1[:], 1.0)
        nc.gpsimd.tensor_tensor(bv[:, :, 0:32], hv[:],
                                av.to_broadcast((128, NJT, D)), op=MUL)
        nc.gpsimd.tensor_tensor(bv[:, :, 32:64], hv[:],
                                aEv.to_broadcast((128, NJT, D)), op=MUL)
        nc.gpsimd.tensor_tensor(bv[:, :, 64:96], bv[:, :, 98:130],
                                bv[:, :, 0:32], op=MUL)
        nc.gpsimd.tensor_copy(bv[:, :, 96:97], av[:])
        nc.gpsimd.tensor_copy(bv[:, :, 97:98], ejv[:])
        onev = one1.rearrange("p (t o) -> p t o", o=1)
        nc.gpsimd.tensor_copy(bv[:, :, 130:131],
                              onev.to_broadcast((128, NJT, 1)))

        # ---------------- scores ---------------------------------------
        sall = sps.tile([128, NJT * RPC], fp32, tag="sall")
        for jt in range(NJT):
            nc.tensor.matmul(sall[:, jt * RPC:(jt + 1) * RPC],
                             qT[:, jt * 128:(jt + 1) * 128],
                             kT[:], start=True, stop=True)

        # ---------------- msk -> X -> M4 pipeline ----------------------
        msk = sb.tile([128, NJT * RPC], fp32, tag="msk")
        X = sb.tile([128, NJT * RPC], bf16, tag="X")
        M4 = keep.tile([128, NJT * RPC], bf16, tag="M4")

        def mskop(jt):
            sl = slice(jt * RPC, (jt + 1) * RPC)
            nc.vector.tensor_tensor(msk[:, sl], at_sb[:, sl], sall[:, sl],
                                    op=MUL)

        def xoph(hh):
            sl = slice(hh * 2 * RPC, (hh + 1) * 2 * RPC)
            nc.scalar.activation(X[:, sl], msk[:, sl], AF.Exp, bias=zero1[:])

        def m4oph(hh):
            sl = slice(hh * 2 * RPC, (hh + 1) * 2 * RPC)
            nc.vector.tensor_tensor(M4[:, sl], X[:, sl], rd_sb[:, sl], op=MUL)

        mskop(0)
        mskop(1)
        xoph(0)
        mskop(2)
        m4oph(0)
        mskop(3)
        xoph(1)
        m4oph(1)

        # ---------------- accumulation matmuls -------------------------
        # one accumulator per i-tile, each in its OWN 2KB PSUM bank —
        # interleaved open accumulation groups in one bank corrupt results
        BNK = 512
        ppA = fps.tile([128, NIT * BNK], fp32, tag="ppA")
        ppAv = ppA.rearrange("p (t c) -> p t c", c=BNK)
        for jt in range(NJT):
            st, sp = (jt == 0), (jt == NJT - 1)
            for it in range(NIT):
                nc.tensor.matmul(
                    ppA[:, it * BNK:it * BNK + GW],
                    M4[:, jt * RPC + it * 128:jt * RPC + (it + 1) * 128],
                    bv[:, jt, :], start=st, stop=sp)

        # ---------------- finals ---------------------------------------
        li = fin.tile([128, NIT, D], fp32, tag="li")
        nc.scalar.activation(li[:], hiv[:], AF.Ln, bias=eps1[:])

        # Pool cannot read PSUM: ACT copies the accumulators to SBUF (for
        # the Pool branch) while DVE starts immediately on PSUM-direct views.
        gA = fin.tile([128, NIT, GW], fp32, tag="gA")
        gAv2 = gA.rearrange("p t c -> p (t c)")
        nc.scalar.activation(gA[:], ppAv[:, :, 0:GW], AF.Identity,
                             bias=zero1[:], scale=1.0)
        # DVE branch reads PSUM directly; Pool branch reads the gA copy
        G3hr = ppAv[:, :, 0:D]
        G3Ehr = ppAv[:, :, D:2 * D]
        G3Lhr = gA[:, :, 0:D]          # Pool-side view of G3hr
        G3Lhr_p = gA[:, :, 2 * D:3 * D]
        r3r = ppAv[:, :, 96:97]
        G4E = ppAv[:, :, 97:98]
        G4L = gA[:, :, 98:130]
        r4 = ppAv[:, :, 130:131]
        r4_p = gA[:, :, 130:131]

        bvw = b_sb2.rearrange("p (t o) -> p t o", o=1)
        eivw = ei_sb.rearrange("p (t o) -> p t o", o=1)

        def bc(ap):
            return ap.to_broadcast((128, NIT, D))

        # row scalars
        r3b = fin.tile([128, NIT], fp32, tag="r3b")
        r3bv = r3b.rearrange("p (t o) -> p t o", o=1)
        nc.vector.tensor_tensor(r3bv[:], r3r, bvw[:], op=MUL)
        s_all = fin.tile([128, NIT], fp32, tag="s_all")
        svw = s_all.rearrange("p (t o) -> p t o", o=1)
        nc.vector.tensor_tensor(svw[:], r3bv[:], r4, op=ADD)
        invs = fin.tile([128, NIT], fp32, tag="invs")
        nc.vector.reciprocal(invs[:], s_all[:])
        m1 = fin.tile([128, NIT], fp32, tag="m1")
        m1v = m1.rearrange("p (t o) -> p t o", o=1)
        nc.vector.tensor_tensor(m1v[:], eivw[:], r4, op=MUL)
        u_all = fin.tile([128, NIT], fp32, tag="u_all")
        uv = u_all.rearrange("p (t o) -> p t o", o=1)
        nc.vector.tensor_sub(uv[:], G4E, m1v[:])

        # ACT takes the per-tile scalar multiplies v2 = hi*u, v3 = hi*r4
        v2 = fin.tile([128, NIT, D], fp32, tag="v2")
        v3 = fin.tile([128, NIT, D], fp32, tag="v3")
        hiv2 = big2_sb[:, 128:192]
        for it in range(NIT):
            nc.scalar.activation(v2[:, it, :], hiv2[:, it * D:(it + 1) * D],
                                 AF.Identity, bias=zero1[:],
                                 scale=u_all[:, it:it + 1])
            nc.scalar.activation(v3[:, it, :], hiv2[:, it * D:(it + 1) * D],
                                 AF.Identity, bias=zero1[:],
                                 scale=gA.rearrange("p t c -> p (t c)")[
                                     :, it * GW + 130:it * GW + 131])
        p2 = fin.tile([128, NIT, D], fp32, tag="p2")
        nc.vector.tensor_mul(p2[:], li[:], v3[:])

        # branch A (DVE): t12 = b*(G3Ehr - Ei*G3hr) + hi*u
        v1 = fin.tile([128, NIT, D], fp32, tag="v1")
        nc.vector.tensor_mul(v1[:], bc(eivw), G3hr)
        t1r = fin.tile([128, NIT, D], fp32, tag="t1r")
        nc.vector.tensor_sub(t1r[:], G3Ehr, v1[:])
        t1 = fin.tile([128, NIT, D], fp32, tag="t1")
        nc.vector.tensor_mul(t1[:], t1r[:], bc(bvw))
        t12 = fin.tile([128, NIT, D], fp32, tag="t12")
        nc.vector.tensor_add(t12[:], t1[:], v2[:])

        # branch B (Pool): e1 = b*(G3Lhr - Li*G3hr); q = hi*G4L
        z1 = fin.tile([128, NIT, D], fp32, tag="z1")
        nc.gpsimd.tensor_tensor(z1[:], li[:], G3Lhr, op=MUL)
        e1r = fin.tile([128, NIT, D], fp32, tag="e1r")
        nc.gpsimd.tensor_tensor(e1r[:], G3Lhr_p, z1[:], op=SUB)
        e1 = fin.tile([128, NIT, D], fp32, tag="e1")
        nc.gpsimd.tensor_tensor(e1[:], e1r[:], bc(bvw), op=MUL)
        q = fin.tile([128, NIT, D], fp32, tag="q")
        nc.gpsimd.tensor_tensor(q[:], hiv[:], G4L, op=MUL)

        # join (DVE)
        e2a = fin.tile([128, NIT, D], fp32, tag="e2a")
        nc.vector.tensor_add(e2a[:], e1[:], q[:])
        e2 = fin.tile([128, NIT, D], fp32, tag="e2")
        nc.vector.tensor_sub(e2[:], e2a[:], p2[:])
        bt = fin.tile([128, NIT, D], fp32, tag="bt")
        bbv = betab.rearrange("p (t d) -> p t d", t=1).to_broadcast(
            (128, NIT, D))
        nc.vector.tensor_mul(bt[:], e2[:], bbv)
        pre = fin.tile([128, NIT, D], fp32, tag="pre")
        nc.vector.tensor_add(pre[:], t12[:], bt[:])
        res = fin.tile([128, NIT, D], fp32, tag="res")
        iv = invs.rearrange("p (t o) -> p t o", o=1)
        nc.vector.tensor_mul(res[:], pre[:], bc(iv))
        nc.sync.dma_start(out[:], res.rearrange("p t d -> p (t d)"))

    nc.compile()
    return nc


def _get_program():
    if "nc" not in _CACHE:
        _CACHE["nc"] = _build_program()
    return _CACHE["nc"]


def make_in_maps(h, pe, E, A, Wk, bk, Wq, bq, beta):
    import ml_dtypes
    f = lambda x: np.ascontiguousarray(np.asarray(x, dtype=np.float32))
    h, pe, E, A = f(h), f(pe), f(E), f(A)
    Wk, bk, Wq, bq, beta = f(Wk), f(bk), f(Wq), f(bq), f(beta)
    in_maps = []
    for c in range(NCORES):
        b, r = c // 2, c % 2
        isl = slice(r * RPC, (r + 1) * RPC)
        smalls = np.zeros((128, 20), np.float32)
        smalls[:, 0:NJT] = E.reshape(NJT, 128).T
        smalls[:, NJT:NJT + NIT] = E[isl].reshape(NIT, 128).T
        smalls[0:D, 6] = bk
        smalls[0:D, 7] = bq
        smalls[:, 8:12] = np.exp(KSH * E).reshape(NJT, 128).T
        smalls[:, 12:14] = np.exp(-KSH * E[isl]).reshape(NIT, 128).T
        smalls[:, 14:18] = (np.exp(KSH * E) * E).reshape(NJT, 128).T
        peR = np.zeros((PED, 832), np.float32)
        peR[:, 0:512] = pe[b].T
        peR[:, 512:544] = Wk
        peR[:, 544:576] = Wq
        peR[:, 576:832] = pe[b, isl].T
        atp = A[isl].T.reshape(NJT, 128, RPC).transpose(1, 0, 2)
        at8 = f(atp.reshape(128, NJT * RPC))
        hjp = h[b].reshape(NJT, 128, D).transpose(1, 0, 2)
        hip = h[b, isl].reshape(NIT, 128, D).transpose(1, 0, 2)
        big2 = np.zeros((128, 224), np.float32)
        big2[:, 0:128] = hjp.reshape(128, NJT * D)
        big2[:, 128:192] = hip.reshape(128, NIT * D)
        big2[:, 192:224] = np.broadcast_to(beta, (128, D))
        # rd[j, i] = 1 - sigmoid(10 (E_j - E_i)), layout [p, (t i)] like AT
        ezt = np.exp(KSH * (E[:, None] - E[None, isl]))      # [j, i]
        rdf = (1.0 / (1.0 + ezt)).astype(np.float32)
        rdp = rdf.reshape(NJT, 128, RPC).transpose(1, 0, 2)
        rdT = np.ascontiguousarray(
            rdp.reshape(128, NJT * RPC)).astype(ml_dtypes.bfloat16)
        in_maps.append({
            "peR": peR,
            "smalls": smalls,
            "at8": at8,
            "big2": big2,
            "rdT": rdT,
        })
    return in_maps


def gather(results):
    out = np.empty((B, N, D), np.float32)
    for c in range(NCORES):
        b, r = c // 2, c % 2
        o = results[c]["out"].reshape(128, NIT, D).transpose(1, 0, 2)
        out[b, r * RPC:(r + 1) * RPC] = o.reshape(RPC, D)
    return out


def _axon_reset():
    try:
        import ctypes
        import jax
        lib = ctypes.CDLL("/opt/axon/libaxon_pjrt.so")
        lib.axon_reset.restype = ctypes.c_int64
        jax.devices()
        lib.axon_reset()
    except Exception:
        pass


def kernel(t=None, h=None, pe=None, E=None, A=None, Wk=None, bk=None,
           Wq=None, bq=None, beta=None, **_unused):
    from concourse.bass_utils import run_bass_kernel_spmd
    nc = _get_program()
    in_maps = make_in_maps(h, pe, E, A, Wk, bk, Wq, bq, beta)
    try:
        res = run_bass_kernel_spmd(nc, in_maps, list(range(NCORES)))
    except Exception:
        # a previously wedged NeuronCore shows up as an opaque runtime
        # error on the first execute — reset the device once and retry
        _axon_reset()
        import time as _time
        _time.sleep(2)
        res = run_bass_kernel_spmd(nc, in_maps, list(range(NCORES)))
    return gather(res.results)
